# revision 93
# baseline (speedup 1.0000x reference)
"""Trainium2 Bass kernel for MQA cross-attention (nn_CrossAttention).

Reference computation (fp32):
    q = (x @ Wq).reshape(b, n, 16, 128).transpose(0,2,1,3) * 128**-0.5
    sim = q @ k^T   (k/v shared across heads, MQA)
    out = softmax(sim) @ v
    y = out.merge_heads @ Wo

Sharding: pure sequence-parallel across 8 cores. Each core gets 256 rows
of x per batch (512 rows total), full Wq/Wo/k/v, and produces its 512 rows
of the output. No collectives, no host-side reduction.

This revision (vs the 259.3us fp32r baseline):
  * All matmul operands bf16 (host-cast): same PE rate as fp32r (1.0
    cycle/row) but half the DMA bytes and SBUF footprint. Measured end-to-end
    error of the full bf16 pipeline vs the fp32 reference: 5.4e-3 (gate 2e-2).
  * Host-side layouts make every DMA line >=512B contiguous per partition
    (the cost model doubles descriptor latency below 512B).
  * Batch-outer unit order: all 16 heads of batch 0 finish halfway through,
    so batch-0's Wo-projection (a pure-PE 27us block) interleaves into
    batch-1's attention units, which are otherwise ACT-exp-limited
    (per unit: ACT ~8.0us > PE-attention ~6.8us).
  * Filler matmuls (q-projection early, Wo-projection late) are emitted
    inside each attention unit at 2-5 matmuls per j-group so the PE wait
    queue always has independent work behind a stalled attn matmul; the
    attn@v matmuls are software-pipelined one j-group behind their exp.
  * Startup: q-projection of heads 0/1 is et-interleaved against the
    streaming wq/xt DMA chunks (8 matmuls per 4-et chunk keeps PE fed);
    wq0/wq1 column chunks are woven between xt chunks so both heads'
    weights and activations arrive just-in-time on the serial DMA wire.
  * Rowsum partials accumulate in bf16 on DVE (2x mode); denominator
    tail (fold + partition all-reduce + reciprocal) stays fp32.
  * o stored as bf16 (host upcasts); the final output tile is split into
    4x128-column groups to shorten the copy+store+drain tail.
"""

import sys
import numpy as np
import ml_dtypes

for _p in ("/opt/trn_rl_repo", "/root/.axon_site/_ro/trn_rl_repo"):
    if _p not in sys.path:
        sys.path.append(_p)

import concourse.bass as bass  # noqa: E402
import concourse.mybir as mybir  # noqa: E402
import concourse.tile as tile  # noqa: E402
from concourse import bacc, bass_isa  # noqa: E402
from concourse.bass_utils import run_bass_kernel_spmd  # noqa: E402

F32 = mybir.dt.float32
BF16 = mybir.dt.bfloat16
NP_BF16 = ml_dtypes.bfloat16

B = 2
N = 2048          # query length (global)
J = 2048          # kv length
E = 2048          # model dim
HEADS = 16
DH = 128          # head dim
NCORES = 8
NC_ROWS = N // NCORES        # 256 query rows per core per batch
R = B * NC_ROWS              # 512 rows per core, col = b*NC_ROWS + i
ET = E // 128                # 16 e-tiles
FT = HEADS                   # 16 f-tiles (one per head, DH == 128)
JT = J // 128                # 16 j-tiles
JG = JT // 2                 # 8 j-groups (2 tiles each)
SCALE = float(DH) ** -0.5

_CACHE = {}


def _build(reps: int = 1):
    nc = bacc.Bacc(name=f"mqa_xattn_r{reps}")
    xt_d = nc.declare_dram_parameter("xt", [128, ET, R], BF16, isOutput=False)
    wq_d = nc.declare_dram_parameter("wq", [HEADS, 128, ET * 128], BF16,
                                     isOutput=False)
    kt_d = nc.declare_dram_parameter("kt", [128, B, J], BF16, isOutput=False)
    v_d = nc.declare_dram_parameter("v", [128, B, JT, DH], BF16,
                                    isOutput=False)
    wo_d = nc.declare_dram_parameter("wo", [FT, 128, E], BF16, isOutput=False)
    o_d = nc.declare_dram_parameter("o", [R, E], BF16, isOutput=True)

    with tile.TileContext(nc) as tc:
        for _ in range(reps):
            _emit_once(nc, tc, xt_d, wq_d, kt_d, v_d, wo_d, o_d)

    nc.compile()
    return nc


def _emit_once(nc, tc, xt_d, wq_d, kt_d, v_d, wo_d, o_d):
    with tc.tile_pool(name="persist", bufs=1) as pp, \
         tc.tile_pool(name="wq_pool", bufs=4) as wqp, \
         tc.tile_pool(name="es_pool", bufs=6) as esp, \
         tc.tile_pool(name="rs_pool", bufs=3) as rsp, \
         tc.tile_pool(name="ost_pool", bufs=6) as ostp, \
         tc.tile_pool(name="sg_ps", bufs=2, space="PSUM") as sg_ps, \
         tc.tile_pool(name="acc_ps", bufs=2, space="PSUM") as acc_ps, \
         tc.tile_pool(name="pj_ps", bufs=2, space="PSUM") as pj_ps:
        kt_sb = pp.tile([128, B, J], BF16)
        v_sb = pp.tile([128, B, JT, DH], BF16)
        xt_sb = pp.tile([128, ET, R], BF16)
        qt_all = pp.tile([128, HEADS, R], BF16)
        # free layout: [b][h][i] with i contiguous per head
        outn_all = pp.tile([128, B, FT * NC_ROWS], BF16)
        wo_sb = pp.tile([128, FT, E], BF16)

        # ---- startup DMA order (SP queue, serial DMA engine model):
        # xt/wq-h0 interleaved finely so qproj h0 starts ~2.5us in; then the
        # rest of xt, wq h1..h3, k/v in batch order.
        def load_wq(h):
            t = wqp.tile([128, ET * 128], BF16, tag="wq", name=f"wq{h}")
            nc.sync.dma_start(t[:], wq_d[h])
            return t

        wq_tiles = {}
        wq_tiles[0] = wqp.tile([128, ET * 128], BF16, tag="wq", name="wq0")
        wq_tiles[1] = wqp.tile([128, ET * 128], BF16, tag="wq", name="wq1")
        # h0/h1 weight columns + xt streamed in 4-et blocks, matching the
        # et-interleaved first qproj pair below (8 matmuls per xt chunk)
        nc.sync.dma_start(wq_tiles[0][:, 0:512], wq_d[0, :, 0:512])
        nc.sync.dma_start(xt_sb[:, 0:4, :], xt_d[:, 0:4, :])
        nc.sync.dma_start(wq_tiles[1][:, 0:1024], wq_d[1, :, 0:1024])
        nc.sync.dma_start(xt_sb[:, 4:8, :], xt_d[:, 4:8, :])
        nc.sync.dma_start(wq_tiles[0][:, 512:1024], wq_d[0, :, 512:1024])
        nc.sync.dma_start(wq_tiles[1][:, 1024:2048], wq_d[1, :, 1024:2048])
        nc.sync.dma_start(xt_sb[:, 8:12, :], xt_d[:, 8:12, :])
        nc.sync.dma_start(wq_tiles[0][:, 1024:1536], wq_d[0, :, 1024:1536])
        nc.sync.dma_start(xt_sb[:, 12:16, :], xt_d[:, 12:16, :])
        nc.sync.dma_start(wq_tiles[0][:, 1536:2048], wq_d[0, :, 1536:2048])
        wq_tiles[2] = load_wq(2)
        nc.sync.dma_start(kt_sb[:, 0, 0:1024], kt_d[:, 0, 0:1024])
        nc.sync.dma_start(v_sb[:, 0, 0:8], v_d[:, 0, 0:8])
        nc.sync.dma_start(kt_sb[:, 0, 1024:2048], kt_d[:, 0, 1024:2048])
        nc.sync.dma_start(v_sb[:, 0, 8:16], v_d[:, 0, 8:16])
        wq_tiles[3] = load_wq(3)
        nc.sync.dma_start(kt_sb[:, 1, :], kt_d[:, 1, :])
        nc.sync.dma_start(v_sb[:, 1], v_d[:, 1])

        def load_wo(ft):
            nc.sync.dma_start(wo_sb[:, ft, :], wo_d[ft])

        # ---- filler task machinery: a generator yielding matmul thunks ----
        def qproj_steps(h):
            """16 accumulating matmuls + 1 copy for head h's q projection.
            h2 takes a free acc-pool bank: at its emission point both pj
            banks are WAR-blocked on the h0/h1 qt copies."""
            wq_sb = wq_tiles.pop(h)
            pool = acc_ps if h == 2 else pj_ps
            q_ps = pool.tile([128, R], F32, tag="acc" if h == 2 else "pj",
                             name=f"qps{h}")
            for et in range(ET):
                yield lambda et=et, q_ps=q_ps, wq_sb=wq_sb: nc.tensor.matmul(
                    q_ps[:], wq_sb[:, et * 128:(et + 1) * 128],
                    xt_sb[:, et, :], start=(et == 0), stop=(et == ET - 1))
            def fin(q_ps=q_ps, h=h):
                with nc.allow_low_precision(reason="bf16 qt"):
                    nc.vector.tensor_copy(qt_all[:, h, :], q_ps[:])
                if h + 4 < HEADS:
                    wq_tiles[h + 4] = load_wq(h + 4)
            yield fin

        def woproj_steps(b, rt, ec, width=512, widths=None,
                         last_on_act=False):
            """Accumulating matmuls + copy + store for one output tile.
            width<512 splits the tile into independent column groups so the
            final store chain is short (drain-tail latency)."""
            if widths is None:
                widths = [width] * (512 // width)
            c0 = 0
            for width in widths:
                o_ps = pj_ps.tile([128, 512], F32, tag="pj",
                                  name=f"ops{b}{rt}{ec}{c0}")
                for ft in range(FT):
                    i0 = ft * NC_ROWS + rt * 128
                    yield lambda ft=ft, o_ps=o_ps, c0=c0, width=width: \
                        nc.tensor.matmul(
                        o_ps[:, 0:width], outn_all[:, b, i0:i0 + 128],
                        wo_sb[:, ft, ec * 512 + c0:ec * 512 + c0 + width],
                        start=(ft == 0), stop=(ft == FT - 1))
                def fin(o_ps=o_ps, c0=c0, width=width):
                    o_sb = ostp.tile([128, 512], BF16, tag="ost",
                                     name=f"osb{b}{rt}{ec}{c0}")
                    with nc.allow_low_precision(reason="bf16 out"):
                        nc.vector.tensor_copy(o_sb[:, 0:width],
                                              o_ps[:, 0:width])
                    nc.sync.dma_start(
                        o_d[b * NC_ROWS + rt * 128:
                            b * NC_ROWS + (rt + 1) * 128,
                            ec * 512 + c0:ec * 512 + c0 + width],
                        o_sb[:, 0:width])
                c0 += width
                yield fin

        filler = []  # list of generators, consumed front to back

        def run_filler(n):
            done = 0
            while filler and done < n:
                try:
                    next(filler[0])()
                    done += 1
                except StopIteration:
                    filler.pop(0)

        # ---- one attention unit: 2 heads x 256 rows x full J, batch b ----
        # Returns a closure emitting the unit's tail (last av pair +
        # softmax-denominator chain + normalize). The caller invokes it
        # inside the NEXT unit after its first sim/exp, so the last av never
        # sits exposed behind its exp's ACT latency at the unit boundary.
        def attn_unit(hp, b, fills, prev_tail=None):
            qt_pair = qt_all[:, 2 * hp:2 * hp + 2,
                             b * NC_ROWS:(b + 1) * NC_ROWS]
            s1024 = rsp.tile([128, 1024], BF16, tag="s1024")
            acch = []  # allocated at jg0 AFTER prev_tail so the WAR on the
            # 2-ago unit's acc covers its (just-emitted) normalize read

            def av(jg, es):
                for kk in range(2):
                    jt = jg * 2 + kk
                    nc.tensor.matmul(acch[0][:], v_sb[:, b, jt, :],
                                     es[:, kk * 512:(kk + 1) * 512],
                                     start=(jt == 0), stop=(jt == JT - 1))

            pend = []  # (jg, es): av runs two j-groups behind its exp
            for jg in range(JG):
                sg = sg_ps.tile([128, 1024], F32, tag="sg")
                for kk in range(2):
                    jt = jg * 2 + kk
                    nc.tensor.matmul(
                        sg[:, kk * 512:(kk + 1) * 512],
                        kt_sb[:, b, jt * 128:(jt + 1) * 128],
                        qt_pair, start=True, stop=True)
                es = esp.tile([128, 1024], BF16, tag="es")
                nc.scalar.activation(
                    es[:], sg[:], mybir.ActivationFunctionType.Exp,
                    scale=SCALE)
                if jg == 0:
                    if prev_tail is not None:
                        prev_tail()
                    acch.append(acc_ps.tile([128, 512], F32, tag="acc",
                                            name=f"acc{hp}{b}"))
                run_filler(fills[jg])
                if len(pend) >= 2:
                    av(*pend.pop(0))
                pend.append((jg, es))
                with nc.allow_low_precision(reason="bf16 rowsum"):
                    if jg == 0:
                        nc.vector.tensor_copy(s1024[:], es[:])
                    else:
                        nc.vector.tensor_add(s1024[:], s1024[:], es[:])

            def tail():
                for p in pend:
                    av(*p)
                # softmax-denominator chain (fp32): fold -> partition
                # all-reduce -> reciprocal -> normalize (writes bf16 outn)
                s512 = rsp.tile([128, 512], F32, tag="s512")
                sB = rsp.tile([128, 512], F32, tag="sB")
                rb_sb = rsp.tile([128, 512], F32, tag="rbs")
                with nc.allow_low_precision(reason="fp32 from bf16 partials"):
                    nc.vector.tensor_add(s512[:], s1024[:, 0:512],
                                         s1024[:, 512:1024])
                    nc.gpsimd.partition_all_reduce(
                        sB[:], s512[:], channels=128,
                        reduce_op=bass_isa.ReduceOp.add)
                    nc.vector.reciprocal(rb_sb[:], sB[:])
                    nc.vector.tensor_mul(
                        outn_all[:, b,
                                 2 * hp * NC_ROWS:(2 * hp + 2) * NC_ROWS],
                        acch[0][:], rb_sb[:])
            return tail

        # ---- program ----
        # first qproj pair, et-interleaved with the startup DMA stream
        q_ps01 = []
        for h in (0, 1):
            q_ps01.append(pj_ps.tile([128, R], F32, tag="pj", name=f"qps{h}"))
        for et in range(ET):
            for h in (0, 1):
                nc.tensor.matmul(q_ps01[h][:],
                                 wq_tiles[h][:, et * 128:(et + 1) * 128],
                                 xt_sb[:, et, :],
                                 start=(et == 0), stop=(et == ET - 1))
        # copies on different engines so they run in parallel
        wq_tiles.pop(0)
        wq_tiles.pop(1)
        with nc.allow_low_precision(reason="bf16 qt"):
            nc.scalar.copy(qt_all[:, 0, :], q_ps01[0][:])
            nc.vector.tensor_copy(qt_all[:, 1, :], q_ps01[1][:])
        wq_tiles[4] = load_wq(4)
        wq_tiles[5] = load_wq(5)
        tail = None
        appended = set()

        def ensure_pair(p):
            # queue qproj pair p; queuing one pair ahead keeps the filler
            # from running dry at jg7 (its steps pull forward FIFO)
            if p < HEADS // 2 and p not in appended:
                appended.add(p)
                filler.append(qproj_steps(2 * p))
                filler.append(qproj_steps(2 * p + 1))

        for hp in range(HEADS // 2):          # batch 0, qproj filler
            ensure_pair(hp + 1)
            ensure_pair(hp + 2)
            load_wo(2 * hp)
            load_wo(2 * hp + 1)
            tail = attn_unit(hp, 0, fills=[6, 6, 5, 5, 5, 5, 4, 4], prev_tail=tail)
        for hp in range(HEADS // 2):          # batch 1, woproj-b0 filler
            rt, ec = divmod(hp, 4)
            filler.append(woproj_steps(0, rt, ec))
            tail = attn_unit(hp, 1, fills=[2] * 8, prev_tail=tail)
        # tail: batch-1 Wo projection, pure PE; last tile in narrow column
        # groups so the final copy+store+drain chain is short. The first two
        # ft matmuls (heads 0/1 only) run before the last unit's deferred
        # tail so its av pair is not exposed behind the final exp.
        for hp in range(HEADS // 2):
            rt, ec = divmod(hp, 4)
            filler.append(woproj_steps(
                1, rt, ec,
                widths=[128] * 4 if hp == 7 else [512]))
        run_filler(2)
        tail()
        run_filler(10000)


def _get_nc(reps: int = 1):
    if reps not in _CACHE:
        _CACHE[reps] = _build(reps)
    return _CACHE[reps]


def _make_in_maps(x, k, v, Wq, Wo):
    # kt[d, b, j] = k[b, j, d]
    kt = np.ascontiguousarray(k.transpose(2, 0, 1)).astype(NP_BF16)
    # v_r[p, b, jt, d] = v[b, jt*128+p, d]
    v_r = np.ascontiguousarray(
        v.reshape(B, JT, 128, DH).transpose(2, 0, 1, 3)).astype(NP_BF16)
    # wq_r[h, p, et*128+f] = Wq[et*128+p, h*128+f]
    wq_r = np.ascontiguousarray(
        Wq.reshape(ET, 128, HEADS, DH).transpose(2, 1, 0, 3).reshape(
            HEADS, 128, ET * 128)).astype(NP_BF16)
    # wo_r[ft, p, e] = Wo[ft*128+p, e]
    wo_r = np.ascontiguousarray(Wo.reshape(FT, 128, E)).astype(NP_BF16)
    in_maps = []
    for c in range(NCORES):
        xs = x[:, c * NC_ROWS:(c + 1) * NC_ROWS, :]  # [B, 256, E]
        # xt[p, et, r] = xs[b(r), i(r), et*128+p]
        xt = np.ascontiguousarray(
            xs.reshape(B * NC_ROWS, ET, 128).transpose(2, 1, 0)).astype(
                NP_BF16)
        in_maps.append({"xt": xt, "kt": kt, "v": v_r, "wq": wq_r, "wo": wo_r})
    return in_maps


def run_on_device(x, k, v, Wq, Wo, reps: int = 1):
    nc = _get_nc(reps)
    in_maps = _make_in_maps(x, k, v, Wq, Wo)
    res = run_bass_kernel_spmd(nc, in_maps, list(range(NCORES)))
    parts = [res.results[c]["o"].astype(np.float32).reshape(B, NC_ROWS, E)
             for c in range(NCORES)]
    return np.concatenate(parts, axis=1)


def kernel(x, k, v, Wq, Wo):
    x = np.asarray(x, dtype=np.float32)
    k = np.asarray(k, dtype=np.float32)
    v = np.asarray(v, dtype=np.float32)
    Wq = np.asarray(Wq, dtype=np.float32)
    Wo = np.asarray(Wo, dtype=np.float32)
    return run_on_device(x, k, v, Wq, Wo, reps=1)
